# revision 10
# baseline (speedup 1.0000x reference)
"""Trainium2 Bass kernel for nn_BookRecommendationModel (retrieval_knn).

Model: BERT-style embedding + LayerNorm + one MHA block (torch MHA with
batch_first=False, so attention runs over the N=64 dim with S=512 as batch)
+ KMeans assignment against 10 centroids of the flattened (N, S*H) output.

Sharding: data-parallel over S (512 -> 8 cores x 64). Every stage is then
core-local; each core emits an (11, 64) partial:
  rows 0..9  = G[k, n] = sum_f flat[n, f] * centroid[k, f]   (local f range)
  row  10    = x2[n]   = sum_f flat[n, f]^2                  (local f range)
Host sums partials over cores, adds ||c_k||^2, computes argmin + loss.

On-chip layout strategy per core (4096 tokens, token = s_local*64 + n):
  gather word_emb rows + pos_emb rows (indirect DMA, accumulate) ->
  LayerNorm (tokens on partitions) -> PE transpose to feature-on-partition
  tiles -> per-head QKV matmuls (M=96 avoids the 96-vs-128 straddle) ->
  scores[m,l] per (s, head), exp (no max subtraction: |scores| ~ 2),
  sum-exp via ones-matmul, reciprocal broadcast via rank-1 matmul ->
  AV -> out_proj (features on partitions) -> distance GEMM vs centroids^T.
"""

import os
import sys

for _p in ("/opt/trn_rl_repo", "/root/.axon_site/_ro/trn_rl_repo"):
    if os.path.isdir(_p) and _p not in sys.path:
        sys.path.insert(0, _p)

from contextlib import ExitStack

import numpy as np

import concourse.bacc as bacc
import concourse.bass as bass
import concourse.mybir as mybir
import concourse.tile as tile
from concourse.bass import IndirectOffsetOnAxis
from concourse.bass_utils import run_bass_kernel_spmd
from concourse.masks import make_identity

F32 = mybir.dt.float32
I32 = mybir.dt.int32

N = 64          # attention sequence dim (torch MHA "L")
S = 512         # batch dim of attention
H = 768
NH = 8
HD = 96         # head dim
E3 = 3 * H      # 2304
NCORES = 8
SLOC = S // NCORES          # 64 s-columns per core
NG = SLOC                   # 64 groups of N=64 tokens per core
TOK = NG * N                # 4096 tokens per core
NTT = TOK // 128            # 32 token-tiles
KT = H // 128               # 6 contraction tiles
EB = H // 128               # 6 output feature tiles
LN_EPS = 1e-5
SCALE = 1.0 / np.sqrt(HD)

# chunking: chunk = 256 tokens (4 groups, 2 token-tiles); superchunk = 2 chunks
N_CHUNK = TOK // 256        # 16
N_SUPER = N_CHUNK // 2      # 8
GP_CHUNK = 4                # groups per chunk
DLOC = SLOC * H             # 49152 features per core
NCB = DLOC // 128           # 384 centroid blocks per core
CB_SUPER = NCB // N_SUPER   # 48 blocks per superchunk


def build_program(n_super=N_SUPER):
    """Build the single-core Bass/Tile program (SPMD across 8 cores).

    n_super can be reduced for faster simulator runs (test only).
    """
    nc = bacc.Bacc("TRN2", debug=False)

    n_chunk = 2 * n_super
    n_tt = 2 * n_chunk
    ncb = CB_SUPER * n_super

    # ---- DRAM tensors (host-prearranged layouts) ----
    wemb = nc.dram_tensor("wemb", [30522, H], F32, kind="ExternalInput").ap()
    pemb = nc.dram_tensor("pemb", [S, H], F32, kind="ExternalInput").ap()
    ids = nc.dram_tensor("ids", [128, NTT], I32, kind="ExternalInput").ap()
    pidx = nc.dram_tensor("pidx", [128, NTT], I32, kind="ExternalInput").ap()
    wt = nc.dram_tensor("wt", [128, KT, E3], F32, kind="ExternalInput").ap()
    wot = nc.dram_tensor("wot", [96, NH, H], F32, kind="ExternalInput").ap()
    ct = nc.dram_tensor("ct", [128, NCB, 10], F32, kind="ExternalInput").ap()
    bqk = nc.dram_tensor("bqk", [96, 16], F32, kind="ExternalInput").ap()
    bv = nc.dram_tensor("bv", [1, H], F32, kind="ExternalInput").ap()
    bo = nc.dram_tensor("bo", [128, EB], F32, kind="ExternalInput").ap()
    gb = nc.dram_tensor("gb", [128, 2 * EB], F32, kind="ExternalInput").ap()
    dout = nc.dram_tensor("dout", [11, N], F32, kind="ExternalOutput").ap()

    with tile.TileContext(nc) as tc, ExitStack() as ctx:
        consts = ctx.enter_context(tc.tile_pool(name="consts", bufs=1))
        embp = ctx.enter_context(tc.tile_pool(name="embp", bufs=4))
        xtp = ctx.enter_context(tc.tile_pool(name="xtp", bufs=2))
        qkp = ctx.enter_context(tc.tile_pool(name="qkp", bufs=1))
        vp = ctx.enter_context(tc.tile_pool(name="vp", bufs=4))
        otp = ctx.enter_context(tc.tile_pool(name="otp", bufs=1))
        atp = ctx.enter_context(tc.tile_pool(name="atp", bufs=1))
        smp = ctx.enter_context(tc.tile_pool(name="smp", bufs=3))
        ctp = ctx.enter_context(tc.tile_pool(name="ctp", bufs=2))
        sqp = ctx.enter_context(tc.tile_pool(name="sqp", bufs=4))
        misc = ctx.enter_context(tc.tile_pool(name="misc", bufs=4))

        ps_big = ctx.enter_context(tc.tile_pool(name="ps_big", bufs=2, space="PSUM"))
        ps_v = ctx.enter_context(tc.tile_pool(name="ps_v", bufs=2, space="PSUM"))
        ps_at = ctx.enter_context(tc.tile_pool(name="ps_at", bufs=3, space="PSUM"))
        ps_d = ctx.enter_context(tc.tile_pool(name="ps_d", bufs=1, space="PSUM"))

        # ---- resident constants ----
        wt_sb = consts.tile([128, KT, E3], F32)
        nc.sync.dma_start(out=wt_sb[:], in_=wt)
        wot_sb = consts.tile([96, NH, H], F32)
        nc.sync.dma_start(out=wot_sb[:], in_=wot)
        ids_sb = consts.tile([128, NTT], I32)
        nc.sync.dma_start(out=ids_sb[:], in_=ids)
        pidx_sb = consts.tile([128, NTT], I32)
        nc.sync.dma_start(out=pidx_sb[:], in_=pidx)
        bqk_sb = consts.tile([96, 16], F32)
        nc.sync.dma_start(out=bqk_sb[:], in_=bqk)
        bo_sb = consts.tile([128, EB], F32)
        nc.sync.dma_start(out=bo_sb[:], in_=bo)
        gb_sb = consts.tile([128, 2 * EB], F32)
        nc.sync.dma_start(out=gb_sb[:], in_=gb)
        bv_sb = consts.tile([1, H], F32)
        nc.sync.dma_start(out=bv_sb[:], in_=bv)

        ident = consts.tile([128, 128], F32)
        make_identity(nc, ident[:])
        ones_col = consts.tile([128, 1], F32)
        nc.vector.memset(ones_col[:], 1.0)
        ones_row = consts.tile([1, 128], F32)
        nc.vector.memset(ones_row[:], 1.0)
        eps_t = consts.tile([128, 1], F32)
        nc.vector.memset(eps_t[:], LN_EPS)

        # broadcast v-bias to all 128 partitions via rank-1 matmul
        bv_bc = consts.tile([128, H], F32)
        for half in range(2):
            pv = ps_v.tile([128, 384], F32, tag="psv")
            nc.tensor.matmul(
                pv[:], lhsT=ones_row[:], rhs=bv_sb[:, half * 384:(half + 1) * 384],
                start=True, stop=True)
            nc.vector.tensor_copy(out=bv_bc[:, half * 384:(half + 1) * 384], in_=pv[:])

        # distance accumulator: rows 0..9 = G10, row 32 = x2 (32-aligned
        # partition base — matmul tile_position must be a multiple of 32)
        comb = ps_d.tile([33, N], F32)

        for w in range(n_super):
            oT_sb = otp.tile([96, NH, 512], F32, tag="oT")
            aT_sb = atp.tile([128, EB, 512], F32, tag="aT")
            ct_sb = ctp.tile([128, CB_SUPER, 10], F32, tag="ct")
            nc.sync.dma_start(
                out=ct_sb[:], in_=ct[:, w * CB_SUPER:(w + 1) * CB_SUPER, :])

            for uu in range(2):
                u = 2 * w + uu
                xT_sb = xtp.tile([128, KT, 256], F32, tag="xT")
                v_tiles = []
                emb_tiles = []

                # ---- gather + LayerNorm per token-tile ----
                for tt in range(2):
                    T = 2 * u + tt
                    emb_t = embp.tile([128, H], F32, tag="emb")
                    nc.gpsimd.indirect_dma_start(
                        out=emb_t[:], out_offset=None,
                        in_=wemb,
                        in_offset=IndirectOffsetOnAxis(ap=ids_sb[:, T:T + 1], axis=0))
                    nc.gpsimd.indirect_dma_start(
                        out=emb_t[:], out_offset=None,
                        in_=pemb,
                        in_offset=IndirectOffsetOnAxis(ap=pidx_sb[:, T:T + 1], axis=0),
                        compute_op=mybir.AluOpType.add)
                    emb_tiles.append(emb_t)

                    # LayerNorm stats (3 subgroups of 256)
                    er = emb_t[:].rearrange("p (a b) -> p a b", b=256)
                    stats = misc.tile([128, 3, 6], F32, tag="stats")
                    for sg in range(3):
                        nc.vector.bn_stats(out=stats[:, sg, :], in_=er[:, sg, :])
                    mv = misc.tile([128, 2], F32, tag="mv")
                    nc.vector.bn_aggr(out=mv[:], in_=stats[:])
                    rstd = misc.tile([128, 1], F32, tag="rstd")
                    nc.scalar.activation(
                        out=rstd[:], in_=mv[:, 1:2],
                        func=mybir.ActivationFunctionType.Sqrt,
                        bias=eps_t[:], scale=1.0)
                    nc.vector.reciprocal(out=rstd[:], in_=rstd[:])
                    nc.vector.tensor_scalar(
                        out=emb_t[:], in0=emb_t[:],
                        scalar1=mv[:, 0:1], scalar2=rstd[:],
                        op0=mybir.AluOpType.subtract, op1=mybir.AluOpType.mult)

                    # transpose 6 x [128,128] with gamma/beta fused in epilogue
                    for eb in range(EB):
                        pt = ps_v.tile([128, 384], F32, tag="psv")
                        nc.tensor.transpose(
                            out=pt[:, 0:128],
                            in_=emb_t[:, eb * 128:(eb + 1) * 128],
                            identity=ident[:])
                        nc.vector.tensor_scalar(
                            out=xT_sb[:, eb, tt * 128:(tt + 1) * 128],
                            in0=pt[:, 0:128],
                            scalar1=gb_sb[:, eb:eb + 1],
                            scalar2=gb_sb[:, EB + eb:EB + eb + 1],
                            op0=mybir.AluOpType.mult, op1=mybir.AluOpType.add)

                # ---- Q/K projections: 16 x (M=96, N=256) ----
                qkT_sb = qkp.tile([96, 16, 256], F32, tag="qk")
                for j in range(16):
                    col0 = j * 96 if j < 8 else H + (j - 8) * 96
                    pq = ps_big.tile([128, 512], F32, tag="big")
                    for kt in range(KT):
                        nc.tensor.matmul(
                            pq[0:96, 0:256],
                            lhsT=wt_sb[:, kt, col0:col0 + 96],
                            rhs=xT_sb[:, kt, :],
                            start=(kt == 0), stop=(kt == KT - 1))
                    sc1 = SCALE if j < 8 else 1.0
                    nc.vector.tensor_scalar(
                        out=qkT_sb[:, j, :], in0=pq[0:96, 0:256],
                        scalar1=sc1, scalar2=bqk_sb[:, j:j + 1],
                        op0=mybir.AluOpType.mult, op1=mybir.AluOpType.add)

                # ---- V projection (row layout, per-group tiles so the AV
                # matmul contraction starts at partition 0) ----
                v_tiles = [
                    vp.tile([64, H], F32, tag="v", name=f"v_{u}_{i}")
                    for i in range(4)
                ]
                for tt in range(2):
                    for half in range(2):
                        hcols = slice(half * 384, (half + 1) * 384)
                        pv = ps_v.tile([128, 384], F32, tag="psv")
                        for kt in range(KT):
                            nc.tensor.matmul(
                                pv[:],
                                lhsT=xT_sb[:, kt, tt * 128:(tt + 1) * 128],
                                rhs=wt_sb[:, kt, 2 * H + half * 384:2 * H + (half + 1) * 384],
                                start=(kt == 0), stop=(kt == KT - 1))
                        for sub in range(2):
                            nc.vector.tensor_tensor(
                                out=v_tiles[tt * 2 + sub][:, hcols],
                                in0=pv[sub * 64:(sub + 1) * 64, :],
                                in1=bv_bc[0:64, hcols],
                                op=mybir.AluOpType.add)

                # ---- attention per group ----
                for gl in range(GP_CHUNK):
                    ccols = slice(gl * 64, gl * 64 + 64)
                    sc = ps_at.tile([65, NH, 64], F32, tag="attn")
                    for h in range(NH):
                        nc.tensor.matmul(
                            sc[0:64, h, :],
                            lhsT=qkT_sb[:, 8 + h, ccols],
                            rhs=qkT_sb[:, h, ccols],
                            start=True, stop=True)
                    exp_sc = smp.tile([64, NH, 64], F32, tag="exp")
                    nc.scalar.activation(
                        out=exp_sc[:], in_=sc[0:64, :, :],
                        func=mybir.ActivationFunctionType.Exp)
                    nc.tensor.matmul(
                        sc[64:65, :, :], lhsT=ones_col[0:64, :], rhs=exp_sc[:],
                        start=True, stop=True)
                    recip_row = misc.tile([1, 512], F32, tag="recip")
                    nc.vector.reciprocal(out=recip_row[:], in_=sc[64:65, :, :])
                    rb = ps_at.tile([128, 512], F32, tag="attn")
                    nc.tensor.matmul(
                        rb[:], lhsT=ones_row[:], rhs=recip_row[:],
                        start=True, stop=True)
                    rb_sb = misc.tile([128, 512], F32, tag="rbsb")
                    nc.vector.tensor_copy(out=rb_sb[:], in_=rb[:])

                    po = ps_at.tile([96, NH, 64], F32, tag="attn")
                    vt = v_tiles[gl]
                    for h in range(NH):
                        nc.tensor.matmul(
                            po[:, h, :],
                            lhsT=vt[:, h * HD:(h + 1) * HD],
                            rhs=exp_sc[:, h, :],
                            start=True, stop=True)
                    off = uu * 256 + gl * 64
                    nc.vector.tensor_tensor(
                        out=oT_sb[:, :, off:off + 64],
                        in0=po[:],
                        in1=rb_sb[0:96, :].rearrange("p (h l) -> p h l", h=NH),
                        op=mybir.AluOpType.mult)

            # ---- out_proj at superchunk granularity (N=512) ----
            for eb in range(EB):
                pa = ps_big.tile([128, 512], F32, tag="big")
                for h in range(NH):
                    nc.tensor.matmul(
                        pa[:],
                        lhsT=wot_sb[:, h, eb * 128:(eb + 1) * 128],
                        rhs=oT_sb[:, h, :],
                        start=(h == 0), stop=(h == NH - 1))
                nc.vector.tensor_scalar(
                    out=aT_sb[:, eb, :], in0=pa[:],
                    scalar1=bo_sb[:, eb:eb + 1], scalar2=None,
                    op0=mybir.AluOpType.add)

            # ---- distance GEMM + x2 ----
            for g8 in range(8):
                gcols = slice(g8 * 64, g8 * 64 + 64)
                for eb in range(EB):
                    blk = g8 * EB + eb
                    first = (w == 0 and blk == 0)
                    last = (w == n_super - 1 and blk == 8 * EB - 1)
                    nc.tensor.matmul(
                        comb[0:10, :],
                        lhsT=ct_sb[:, blk, :],
                        rhs=aT_sb[:, eb, gcols],
                        start=first, stop=last, skip_group_check=True)
                    sq = sqp.tile([128, 64], F32, tag="sq")
                    nc.gpsimd.tensor_mul(
                        out=sq[:], in0=aT_sb[:, eb, gcols], in1=aT_sb[:, eb, gcols])
                    nc.tensor.matmul(
                        comb[32:33, :],
                        lhsT=ones_col[:],
                        rhs=sq[:],
                        start=first, stop=last, skip_group_check=True)

        dstage = misc.tile([33, N], F32, tag="dstage")
        nc.vector.tensor_copy(out=dstage[0:10, :], in_=comb[0:10, :])
        nc.vector.tensor_copy(out=dstage[32:33, :], in_=comb[32:33, :])
        nc.sync.dma_start(out=dout[0:10, :], in_=dstage[0:10, :])
        nc.sync.dma_start(out=dout[10:11, :], in_=dstage[32:33, :])

    nc.compile()
    return nc


def host_prep(inputs, n_super=N_SUPER):
    """Build per-core in_maps from the full problem inputs."""
    input_ids = np.asarray(inputs["input_ids"]).astype(np.int32)
    word_emb = np.ascontiguousarray(np.asarray(inputs["word_emb"], np.float32))
    pos_emb = np.ascontiguousarray(np.asarray(inputs["pos_emb"], np.float32))
    ln_gamma = np.asarray(inputs["ln_gamma"], np.float32)
    ln_beta = np.asarray(inputs["ln_beta"], np.float32)
    in_w = np.asarray(inputs["in_proj_w"], np.float32)
    in_b = np.asarray(inputs["in_proj_b"], np.float32)
    out_w = np.asarray(inputs["out_proj_w"], np.float32)
    out_b = np.asarray(inputs["out_proj_b"], np.float32)
    cen = np.asarray(inputs["centroids"], np.float32)

    wt = np.ascontiguousarray(
        in_w.T.reshape(KT, 128, E3).transpose(1, 0, 2))
    wot = np.ascontiguousarray(
        out_w.T.reshape(NH, 96, H).transpose(1, 0, 2))
    bqk = np.empty((96, 16), np.float32)
    for h in range(8):
        bqk[:, h] = in_b[h * 96:(h + 1) * 96] * SCALE
        bqk[:, 8 + h] = in_b[H + h * 96:H + (h + 1) * 96]
    bv = np.ascontiguousarray(in_b[2 * H:][None, :])
    bo = np.ascontiguousarray(out_b.reshape(EB, 128).T)
    gbm = np.concatenate(
        [ln_gamma.reshape(EB, 128).T, ln_beta.reshape(EB, 128).T], axis=1)
    gbm = np.ascontiguousarray(gbm)

    in_maps = []
    for c in range(NCORES):
        # token t*128+p -> (g, n); s = c*SLOC + g
        tokens = np.arange(TOK)
        g = tokens // N
        n = tokens % N
        s = c * SLOC + g
        ids_arr = np.ascontiguousarray(
            input_ids[n, s].reshape(NTT, 128).T.astype(np.int32))
        pidx_arr = np.ascontiguousarray(
            s.reshape(NTT, 128).T.astype(np.int32))
        cslice = cen[:, c * DLOC:(c + 1) * DLOC]
        ct_arr = np.ascontiguousarray(
            cslice.T.reshape(NCB, 128, 10).transpose(1, 0, 2))
        in_maps.append({
            "wemb": word_emb, "pemb": pos_emb,
            "ids": ids_arr, "pidx": pidx_arr,
            "wt": wt, "wot": wot, "ct": ct_arr,
            "bqk": bqk, "bv": bv, "bo": bo, "gb": gbm,
        })
    return in_maps


def postprocess(results, centroids):
    """Combine per-core (11, 64) partials into (cl, loss)."""
    parts = np.stack([np.asarray(r["dout"]) for r in results])  # (8, 11, 64)
    tot = parts.sum(axis=0)
    G = tot[0:10, :]                  # (10, n)
    x2 = tot[10, :]                   # (n,)
    c2 = (np.asarray(centroids, np.float32).astype(np.float64) ** 2).sum(axis=1)
    d = x2[None, :].astype(np.float64) + c2[:, None] - 2.0 * G.astype(np.float64)
    cl = np.argmin(d, axis=0).astype(np.int32)
    loss = np.float32(d[cl, np.arange(N)].sum())
    return cl, loss


_CACHED = {}


def _get_program():
    if "nc" not in _CACHED:
        _CACHED["nc"] = build_program()
    return _CACHED["nc"]


def run(inputs, trace=False, trace_kwargs=None):
    nc = _get_program()
    in_maps = host_prep(inputs)
    res = run_bass_kernel_spmd(
        nc, in_maps, core_ids=list(range(NCORES)), trace=trace,
        **(trace_kwargs or {}))
    cl, loss = postprocess(res.results, inputs["centroids"])
    return cl, loss, res


def kernel(**inputs):
    cl, loss, _ = run(inputs)
    return cl, loss
